# revision 12
# baseline (speedup 1.0000x reference)
"""Trainium2 Bass kernel for a binarized-weight BasicBlock (dense CNN).

Reference computation (all fp32):
    out = clip(bn2(conv3x3(quant(clip(bn1(conv3x3(quant(x), sign(w1))), -1, 1)),
                  sign(w2)) + x), -1, 1)
with quant(v) = round-half-up(v * 128) / 128 and bn in inference form.

Strategy (v2: 1D Winograd F(2,3) along W):
  * Data-parallel: batch 32 is sharded 4 images per NeuronCore across 8 cores.
  * Channels (256) live on partitions as 2 blocks of 128.
  * Each 3-tap row convolution is computed via Winograd F(2,3): per vertical
    tap dy and input block ib, 4 transformed components t_c feed 4 accumulating
    matmuls producing 4 PSUM component planes m_c over 28 column-pairs; the
    inverse transform (even = m0+m1+m2, odd = m1-m2-m3) runs on DVE.  This is
    2/3 the PE work of direct conv (24 vs 36 matmul-columns per output pair).
  * Exactness: activations are integers k = 128*quant(v), |k| <= ~730; the
    W-transform doubles magnitude (<= 2048, fp16-exact); weights are
    2*G*sign(w) in {0,+-1,+-2,+-3} (fp16-exact); PSUM accumulates exact
    integers < 2^24.  The conv itself is bit-exact; BN epilogues use fused
    Relu-chains that may differ from XLA by ~1ulp (well within 2e-2).
  * Quantized activations are stored as even/odd column planes so all
    Winograd forward transforms are unit-stride adds/subs (GpSimd), and
    conv1's even/odd Winograd outputs write straight into conv2's planes.
  * hardtanh clips are folded into two chained Relu activations on the ACT
    engine (clip(z,lo,hi) via 128.5-min / max tricks), eliminating the slow
    DVE MIN,MAX ops; floor() uses the magic-number RNE + is_gt correction.
  * Elementwise work is spread over DVE / ACT / GpSimd so all three stay
    under the ~259us PE matmul floor.
"""

import numpy as np

_N = 32          # full batch
_C = 256         # channels
_H = 56          # height
_W = 56          # width
_NCORES = 8
_EPS = 1e-5

_cache = {}


def _build(n_img, C, H, W):
    """Build + compile the per-core Bass program (SPMD, one NEFF for all cores)."""
    from contextlib import ExitStack

    import concourse.tile as tile
    from concourse import bacc, mybir

    F32 = mybir.dt.float32
    F16 = mybir.dt.float16
    I16 = mybir.dt.int16
    Alu = mybir.AluOpType
    Act = mybir.ActivationFunctionType

    MAGIC = float(3 << 22)  # 1.5 * 2**23: RNE-to-integer for 0 <= z < 2**22

    nblk = C // 128
    NJ = W // 2              # 28 column pairs
    HP = H + 2               # H-padded rows in t planes
    NP = NJ + 1              # 29 cols in e/o planes (incl one pad col)
    RGX = 8                  # quant rowgroup
    ngx = H // RGX
    GRPS = [(0, 16), (16, 16), (32, 16), (48, 8)]  # matmul row groups
    NWC = 24                 # weight tiles per (conv, ob): 4 comps x 3 dy x 2 ib
    NW = 2 * nblk * NWC      # 96

    nc = bacc.Bacc("TRN2", target_bir_lowering=False, debug=False,
                   num_devices=_NCORES)

    x_d = nc.dram_tensor("x", [n_img, C, H * W], F32, kind="ExternalInput")
    w_d = nc.dram_tensor("wq", [128, NW, 128], F16, kind="ExternalInput")
    c_d = nc.dram_tensor("coef", [128, nblk, 6], F32, kind="ExternalInput")
    o_d = nc.dram_tensor("out", [n_img, C, H * W], F32, kind="ExternalOutput")

    with tile.TileContext(nc) as tc, ExitStack() as ctx:
        const = ctx.enter_context(tc.tile_pool(name="const", bufs=1))
        xin = ctx.enter_context(tc.tile_pool(name="xin", bufs=4))
        qpl = ctx.enter_context(tc.tile_pool(name="qpl", bufs=2))
        tcp = ctx.enter_context(tc.tile_pool(name="tcp", bufs=2))
        qtmp = ctx.enter_context(tc.tile_pool(name="qtmp", bufs=2))
        etmp = ctx.enter_context(tc.tile_pool(name="etmp", bufs=2))
        resp = ctx.enter_context(tc.tile_pool(name="resp", bufs=2))
        stg = ctx.enter_context(tc.tile_pool(name="stg", bufs=2))
        psum = ctx.enter_context(tc.tile_pool(name="psum", bufs=7, space="PSUM"))
        warmp = ctx.enter_context(tc.tile_pool(name="warmp", bufs=1,
                                               space="PSUM"))

        # ---- weights: chunked by (conv, ob); mini-chunk first for warmup ----
        wt = const.tile([128, NW, 128], F16)
        nc.sync.dma_start(wt[:, 0:4, :], w_d.ap()[:, 0:4, :])
        ct = const.tile([128, nblk, 6], F32)
        nc.sync.dma_start(ct[:], c_d.ap())

        # ---- image 0 input: rowgroup pieces so quant/tf can start early ----
        def x_dma(i, dst_tag_list):
            xi = x_d.ap()[i].rearrange("(b p) f -> p b f", p=128)
            xg = []
            for g in range(ngx):
                xt = xin.tile([128, nblk, RGX * W], F32, tag="x")
                nc.sync.dma_start(xt[:], xi[:, :, g * RGX * W:(g + 1) * RGX * W])
                xg.append(xt)
            return xg

        xg0 = x_dma(0, None)
        nc.sync.dma_start(wt[:, 4:NWC, :], w_d.ap()[:, 4:NWC, :])
        for cv in range(2):
            for ob in range(nblk):
                if cv == 0 and ob == 0:
                    continue
                ch = (cv * nblk + ob) * NWC
                nc.sync.dma_start(wt[:, ch:ch + NWC, :],
                                  w_d.ap()[:, ch:ch + NWC, :])

        # ---- PE clock warmup ----
        warm = warmp.tile([128, 128], F32)
        for j in range(65):
            nc.tensor.matmul(warm[:], wt[:, 0, :], wt[:, j % 4, :],
                             start=True, stop=True)

        def quant_to_planes(xg, qe, qo):
            """quant rowgroups of x into even/odd integer planes (fp16).

            k = round-half-up(128*x) == RNE(128*x) except exact .5 ties,
            which are vanishingly rare in fp32 (analysed: ~1e-3 rel budget).
            The ACT engine's int16 output convert does the RNE for free.
            """
            for g in range(ngx):
                r0 = g * RGX
                zi = qtmp.tile([128, nblk, RGX * W], I16, tag="qz")
                nc.scalar.activation(zi[:], xg[g][:], Act.Copy,
                                     bias=0.0, scale=128.0)
                z4 = zi[:].rearrange("p b (h w) -> p b h w", w=W)
                nc.vector.tensor_copy(qe[:, :, r0:r0 + RGX, 0:NJ],
                                      z4[:, :, :, 0::2])
                nc.vector.tensor_copy(qo[:, :, r0:r0 + RGX, 1:NP],
                                      z4[:, :, :, 1::2])

        def fwd_transform(qe, qo, tcb):
            """4 Winograd comps from e/o planes into tcb[p, ib, comp, HP, NJ]."""
            nc.vector.memset(tcb[:, :, :, 0, :], 0.0)
            nc.vector.memset(tcb[:, :, :, HP - 1, :], 0.0)
            for (r0, rg) in GRPS:
                rs, rd = slice(r0, r0 + rg), slice(1 + r0, 1 + r0 + rg)
                e0 = qe[:, :, rs, 0:NJ]
                e1 = qe[:, :, rs, 1:NP]
                o0 = qo[:, :, rs, 0:NJ]
                o1 = qo[:, :, rs, 1:NP]
                nc.vector.tensor_tensor(tcb[:, :, 0, rd, :], o0, o1,
                                        Alu.subtract)
                nc.vector.tensor_tensor(tcb[:, :, 1, rd, :], e0, o1, Alu.add)
                nc.vector.tensor_tensor(tcb[:, :, 2, rd, :], o1, e0,
                                        Alu.subtract)
                nc.vector.tensor_tensor(tcb[:, :, 3, rd, :], e0, e1,
                                        Alu.subtract)

        def conv_mms(ps, tcb, cv, ob, comp, r0, rg):
            """6 accumulating matmuls: one Winograd component plane."""
            for dy in range(3):
                for ib in range(nblk):
                    widx = (cv * nblk + ob) * NWC + comp * 6 + dy * 2 + ib
                    rhs = tcb[:, ib, comp, r0 + dy:r0 + dy + rg, :]
                    nc.tensor.matmul(ps[:], wt[:, widx, :], rhs,
                                     start=(dy == 0 and ib == 0),
                                     stop=(dy == 2 and ib == 1))

        def inverse_to(ms, dst_eo, n):
            """dst[:,0,:n] = m0+m1+m2 ; dst[:,1,:n] = m1-m2-m3.

            Engines may read at most one PSUM operand per instruction, so m1
            is staged through SBUF once; every op below has <=1 PSUM input.
            """
            e, o = dst_eo[:, 0, :n], dst_eo[:, 1, :n]
            m0, m1, m2, m3 = (m[:].rearrange("p h w -> p (h w)") for m in ms)
            a = etmp.tile([128, 16 * NJ], F32, tag="m1c")
            nc.vector.tensor_copy(a[:, :n], m1)
            nc.vector.tensor_tensor(e, m2, a[:, :n], Alu.add)       # m1+m2
            nc.vector.tensor_tensor(e, m0, e, Alu.add)              # +m0
            nc.vector.tensor_tensor(o, a[:, :n], m2, Alu.subtract)  # m1-m2
            nc.vector.tensor_tensor(o, o, m3, Alu.subtract)         # -m3

        for i in range(n_img):
            xg = xg0 if i == 0 else x_dma(i, None)

            # quantize input into even/odd planes
            qe1 = qpl.tile([128, nblk, H, NP], F16, tag="q1e")
            qo1 = qpl.tile([128, nblk, H, NP], F16, tag="q1o")
            nc.vector.memset(qe1[:, :, :, NP - 1:NP], 0.0)
            nc.vector.memset(qo1[:, :, :, 0:1], 0.0)
            quant_to_planes(xg, qe1, qo1)

            # conv1 forward transform
            tc1 = tcp.tile([128, nblk, 4, HP, NJ], F16, tag="tc")
            fwd_transform(qe1, qo1, tc1)

            # conv1 matmuls + epilogue -> conv2 input planes
            qe2 = qpl.tile([128, nblk, H, NP], F16, tag="q2e")
            qo2 = qpl.tile([128, nblk, H, NP], F16, tag="q2o")
            nc.vector.memset(qe2[:, :, :, NP - 1:NP], 0.0)
            nc.vector.memset(qo2[:, :, :, 0:1], 0.0)
            for ob in range(nblk):
                for (r0, rg) in GRPS:
                    n = rg * NJ
                    ms = []
                    for comp in range(4):
                        ps = psum.tile([128, rg, NJ], F32, tag="ps")
                        conv_mms(ps, tc1, 0, ob, comp, r0, rg)
                        ms.append(ps)
                    teo = etmp.tile([128, 2, 16 * NJ], F32, tag="teo")
                    inverse_to(ms, teo, n)
                    # r1 = Relu(-z + 128.5), r2 = Relu(-r1 + 256.5)
                    r1 = etmp.tile([128, 2, 16 * NJ], F32, tag="r1")
                    nc.scalar.activation(
                        r1[:, :, :n],
                        teo[:, :, :n],
                        Act.Relu, bias=ct[:, ob, 1:2], scale=ct[:, ob, 0:1])
                    # r2 = Relu(-r1 + 256) -> int16 RNE == q + 128
                    # (floor(y-127.5) = RNE(y-0.5)-127... folded; exact-tie
                    # corner is measure-zero in fp32, within error budget)
                    r2 = etmp.tile([128, 2, 16 * NJ], I16, tag="r2i")
                    nc.scalar.activation(
                        r2[:, :, :n],
                        r1[:, :, :n],
                        Act.Relu, bias=ct[:, ob, 4:5], scale=-1.0)
                    rr = slice(r0, r0 + rg)
                    nc.vector.tensor_scalar(
                        qe2[:, ob, rr, 0:NJ],
                        r2[:, 0, :n].rearrange("p (h w) -> p h w", w=NJ),
                        128.0, None, Alu.subtract)
                    nc.vector.tensor_scalar(
                        qo2[:, ob, rr, 1:NP],
                        r2[:, 1, :n].rearrange("p (h w) -> p h w", w=NJ),
                        128.0, None, Alu.subtract)

            # conv2 forward transform
            tc2 = tcp.tile([128, nblk, 4, HP, NJ], F16, tag="tc")
            fwd_transform(qe2, qo2, tc2)

            # conv2 matmuls + epilogue -> output
            for ob in range(nblk):
                for (r0, rg) in GRPS:
                    n = rg * NJ
                    ms = []
                    for comp in range(4):
                        ps = psum.tile([128, rg, NJ], F32, tag="ps")
                        conv_mms(ps, tc2, 1, ob, comp, r0, rg)
                        ms.append(ps)
                    teo = etmp.tile([128, 2, 16 * NJ], F32, tag="teo")
                    inverse_to(ms, teo, n)
                    # s = conv/256 + residual (even/odd columns)
                    res = resp.tile([128, 16 * W], F32, tag="res")
                    nc.sync.dma_start(
                        res[:, :rg * W],
                        x_d.ap()[i, ob * 128:(ob + 1) * 128,
                                 r0 * W:(r0 + rg) * W])
                    r4 = res[:, :rg * W].rearrange("p (h w) -> p h w", w=W)
                    s = teo
                    nc.vector.scalar_tensor_tensor(
                        s[:, 0, :n].rearrange("p (h w) -> p h w", w=NJ),
                        teo[:, 0, :n].rearrange("p (h w) -> p h w", w=NJ),
                        1.0 / 256.0, r4[:, :, 0::2], Alu.mult, Alu.add)
                    nc.vector.scalar_tensor_tensor(
                        s[:, 1, :n].rearrange("p (h w) -> p h w", w=NJ),
                        teo[:, 1, :n].rearrange("p (h w) -> p h w", w=NJ),
                        1.0 / 256.0, r4[:, :, 1::2], Alu.mult, Alu.add)
                    # r1 = Relu(bn + 1), r2 = Relu(-r1 + 2), out = 1 - r2
                    r1 = etmp.tile([128, 2, 16 * NJ], F32, tag="r1")
                    nc.scalar.activation(
                        r1[:, :, :n],
                        s[:, :, :n],
                        Act.Relu, bias=ct[:, ob, 3:4], scale=ct[:, ob, 2:3])
                    r2 = etmp.tile([128, 2, 16 * NJ], F32, tag="r2")
                    nc.scalar.activation(
                        r2[:, :, :n],
                        r1[:, :, :n],
                        Act.Relu, bias=ct[:, ob, 5:6], scale=-1.0)
                    ot = stg.tile([128, 16, W], F32, tag="ot")
                    nc.scalar.activation(
                        ot[:, :rg, 0::2],
                        r2[:, 0, :n].rearrange("p (h w) -> p h w", w=NJ),
                        Act.Identity, bias=1.0, scale=-1.0)
                    nc.scalar.activation(
                        ot[:, :rg, 1::2],
                        r2[:, 1, :n].rearrange("p (h w) -> p h w", w=NJ),
                        Act.Identity, bias=1.0, scale=-1.0)
                    nc.sync.dma_start(
                        o_d.ap()[i, ob * 128:(ob + 1) * 128,
                                 r0 * W:(r0 + rg) * W],
                        ot[:, :rg, :].rearrange("p h w -> p (h w)"))

    nc.compile()
    return nc


def _get_program(n_img, C, H, W):
    key = (n_img, C, H, W)
    if key not in _cache:
        _cache[key] = _build(n_img, C, H, W)
    return _cache[key]


def _fold_bn(g, b, m, v):
    """Per-channel (inv, bias) in fp32, matching the reference's op sequence."""
    inv = (g.astype(np.float32)
           * (np.float32(1.0) / np.sqrt(v.astype(np.float32)
                                        + np.float32(_EPS))))
    bias = b.astype(np.float32) - m.astype(np.float32) * inv
    return inv.astype(np.float32), bias.astype(np.float32)


def _prep_weights(w1, w2, C):
    """[128, 96, 128] fp16 Winograd lhsT tiles (i on partitions, o on free).

    Component weights are 2*G*sign(w): [2*w0, w0+w1+w2, w0-w1+w2, 2*w2].
    """
    nblk = C // 128
    NWC = 24
    tiles = np.empty((128, 2 * nblk * NWC, 128), np.float16)
    for cv, w in enumerate((w1, w2)):
        wq = np.where(w >= 0, np.float32(1.0), np.float32(-1.0))
        for ob in range(nblk):
            for comp in range(4):
                for dy in range(3):
                    for ib in range(nblk):
                        idx = (cv * nblk + ob) * NWC + comp * 6 + dy * 2 + ib
                        blk = wq[ob * 128:(ob + 1) * 128,
                                 ib * 128:(ib + 1) * 128, dy, :]
                        w0, wm, w2t = blk[:, :, 0], blk[:, :, 1], blk[:, :, 2]
                        if comp == 0:
                            wc = 2.0 * w0
                        elif comp == 1:
                            wc = w0 + wm + w2t
                        elif comp == 2:
                            wc = w0 - wm + w2t
                        else:
                            wc = 2.0 * w2t
                        tiles[:, idx, :] = wc.astype(np.float16).T
    return tiles


def _make_in_maps(x, w1, w2, g1, b1, m1, v1, g2, b2, m2, v2):
    n, C, H, W = x.shape
    n_img = n // _NCORES
    nblk = C // 128

    wq = _prep_weights(np.asarray(w1), np.asarray(w2), C)
    inv1, bias1 = _fold_bn(np.asarray(g1), np.asarray(b1),
                           np.asarray(m1), np.asarray(v1))
    inv2, bias2 = _fold_bn(np.asarray(g2), np.asarray(b2),
                           np.asarray(m2), np.asarray(v2))
    coef = np.empty((128, nblk, 6), np.float32)
    for blk in range(nblk):
        sl = slice(blk * 128, (blk + 1) * 128)
        coef[:, blk, 0] = -0.5 * inv1[sl]
        coef[:, blk, 1] = np.float32(128.0) - np.float32(128.0) * bias1[sl]
        coef[:, blk, 2] = inv2[sl]
        coef[:, blk, 3] = np.float32(1.0) + bias2[sl]
        coef[:, blk, 4] = np.float32(256.0)
        coef[:, blk, 5] = np.float32(2.0)

    xr = np.ascontiguousarray(np.asarray(x).reshape(n, C, H * W),
                              dtype=np.float32)
    return [
        {"x": xr[i * n_img:(i + 1) * n_img], "wq": wq, "coef": coef}
        for i in range(_NCORES)
    ]


def _run(trace=False, **inputs):
    from concourse.bass_utils import run_bass_kernel_spmd

    n, C, H, W = inputs["x"].shape
    nc = _get_program(n // _NCORES, C, H, W)
    in_maps = _make_in_maps(**inputs)
    res = run_bass_kernel_spmd(nc, in_maps, core_ids=list(range(_NCORES)),
                               trace=trace)
    out = np.concatenate([r["out"] for r in res.results], axis=0)
    return out.reshape(n, C, H, W), res


def kernel(x, w1, w2, g1, b1, m1, v1, g2, b2, m2, v2):
    out, _ = _run(x=x, w1=w1, w2=w2, g1=g1, b1=b1, m1=m1, v1=v1,
                  g2=g2, b2=b2, m2=m2, v2=v2)
    return out
